# revision 24
# baseline (speedup 1.0000x reference)
"""Nearest-neighbor tokenizer on 8 Trainium2 NeuronCores.

Math: d2[t,m] = ||x_t||^2 + ||c_m||^2 - 2 x_t.c_m over 65536 tokens x 4096 codes.
out[t] = argmin_m d2 if min d2 <= 0.1 else -1.

Fast path ("screen" program): per token, mind2 = 0.1 - 2*max_m h[t,m] where
h = x.c - ||c||^2/2 - cap_t and cap_t = (||x_t||^2 - 0.1)/2 (over the first DS
dims; partial d2 <= d2 keeps it sound). All-tokens-miss is certified by
max h < 0 with margin. Token blocks are processed in even/odd pairs computed
concurrently by the two 64-row PE tiles into separate PSUM pools; each fp32 is
read exactly once: DVE fused-max-reduces the even block's banks while ACT
fused-relu(+margin)-sums (accum_out) the odd block's banks in parallel. Host
checks the 2x128 per-core outputs; if any token might be within threshold,
falls back to the exact argmin program.

Sharding: data-parallel over tokens. Core c gets batches [2c, 2c+2) ->
a contiguous slab of 8192 tokens; the codebook is replicated.
"""

import os

import numpy as np

B, N, D = 16, 4096, 64
M = 4096
NCORES = 8
TOK = B * N // NCORES          # 8192 tokens per core
NBLK = TOK // 128              # 64 blocks of 128 tokens
NCH = M // 512                 # 8 chunks of 512 codes
CBLK = M // 128                # 32 code blocks
THRESH = 0.1
DS = 63                        # screen dims (63 + appended row = K=64 tiles)
MARGIN = 2.0                   # screen slack (>> bf16 quantization error ~0.6)
FALLBACK_MARGIN = 2.0

_CACHE = {}


def _build_screen():
    """mind2 screen over the first DS dims (partial d2 <= d2, so certifying
    partial mind2 > THRESH is sound). Output per core: screen[0:128] = max
    over this partition's tokens of (max_m g~ - cap), screen[128:256] = sum of
    relu(g~ - cap + MARGIN) over ACT's share. All-clear iff screen[0:128] < -1
    and screen[128:256] == 0.

    The PE on this part is pinned at the 1.2 GHz throttled clock (HAM never
    reaches K=8/8 even under 6.8us of continuous matmuls), so single matmuls
    would cost 218us. The contraction is kept at K=64 (63 dims + appended
    -c2/2 / ones row): the 128x128 xbar transpose packs an EVEN token block
    into xT rows 0:64 and the following ODD block into rows 64:128, and cT
    carries a duplicate of the codes at partitions 64:128, so the two 64-row
    PE tiles compute both blocks of a pair concurrently. Even blocks are
    screened by the DVE max-reduce, odd blocks by the ACT relu-sum — both
    sound."""
    import concourse.bacc as bacc
    import concourse.mybir as mybir
    import concourse.tile as tile
    from contextlib import ExitStack

    fp32 = mybir.dt.float32
    bf16 = mybir.dt.bfloat16
    Alu = mybir.AluOpType
    Act = mybir.ActivationFunctionType
    AX = mybir.AxisListType.X

    nc = bacc.Bacc(
        "TRN2",
        target_bir_lowering=False,
        debug=False,
        enable_asserts=False,
        num_devices=1,
    )

    x_d = nc.dram_tensor("x", (TOK, D), fp32, kind="ExternalInput")
    c_d = nc.dram_tensor("codes", (M, D), fp32, kind="ExternalInput")
    o_d = nc.dram_tensor("screen", (256,), fp32, kind="ExternalOutput")

    ACT_N = 1024               # ACT's per-group drain (2 of 4 PSUM banks)

    with tile.TileContext(nc) as tc, ExitStack() as ctx:
        sb = ctx.enter_context(tc.tile_pool(name="sb", bufs=1))

        NPAIR = NBLK // 2             # 32 even/odd block pairs
        xsb = sb.tile((128, NBLK, D), fp32, tag="xsb")
        csb = sb.tile((128, CBLK, D), fp32, tag="csb")
        # bf16 copies; x: 64 cols per block (63 dims + ones col); codes:
        # 128 cols per block (dims + -c2/2 col, duplicated at cols 64..)
        xbf = sb.tile((128, NBLK, 64), bf16, tag="xbf")
        cbf = sb.tile((128, CBLK, 128), bf16, tag="cbf")
        # xT[*, pr, t]: rows 0:64 = even block of pair pr, rows 64:128 = odd
        xT = sb.tile((128, NPAIR, 128), bf16, tag="xT")
        cT = sb.tile((128, CBLK, 128), bf16, tag="cT")
        xsq = sb.tile((128, NBLK, DS), bf16, tag="xsq")
        csq = sb.tile((128, CBLK, DS), bf16, tag="csq")
        x2 = sb.tile((128, NPAIR, 2), fp32, tag="x2")
        c2 = sb.tile((128, CBLK), fp32, tag="c2")
        cap = sb.tile((128, NPAIR, 2), fp32, tag="cap")  # (x2 - 0.1)/2
        ncb = sb.tile((128, NPAIR, 2), fp32, tag="ncb")  # MARGIN - cap
        gmax = sb.tile((128, NPAIR, 4), fp32, tag="gmax")
        rsum = sb.tile((128, NPAIR * 4), fp32, tag="rsum")
        scr = sb.tile((128, ACT_N), bf16, tag="scr")
        gm2 = sb.tile((128, NPAIR), fp32, tag="gm2")
        fin = sb.tile((128, 2), fp32, tag="fin")

        dma = nc.default_dma_engine
        XCH = 4                       # x pipeline chunks
        XB = NBLK // XCH              # blocks per chunk
        CH2 = CBLK // 2
        # Sync queue: codes halves + x chunk 0 (the setup critical path);
        # x chunks 1-3 issue from the ACT queue after its critical-path ops.
        # Contiguous loads (16KB/partition runs): partition p's "block" r is
        # token 64p+r (codes: 32p+r). The screen is order-invariant, so any
        # consistent token/code <-> (p, r) mapping works; this one makes the
        # DMA fully sequential instead of 256B-strided.
        c_r = c_d[:, :].rearrange("(p r) d -> p r d", p=128)
        x_r = x_d[:, :].rearrange("(p r) d -> p r d", p=128)
        CQ = CBLK // 4
        for h in range(4):
            dma.dma_start(out=csb[:, h * CQ:(h + 1) * CQ, :],
                          in_=c_r[:, h * CQ:(h + 1) * CQ, :])
        dma.dma_start(out=xsb[:, 0:XB, :], in_=x_r[:, 0:XB, :])

        nc.vector.memset(xbf[:, :, DS:DS + 1], 1.0)

        def codes_half(h):
            hs = slice(h * CQ, (h + 1) * CQ)
            nc.vector.tensor_copy(cbf[:, hs, 0:DS], csb[:, hs, 0:DS])
            nc.vector.tensor_copy(cbf[:, hs, 64:64 + DS], csb[:, hs, 0:DS])
            nc.scalar.activation(csq[:, hs, :], csb[:, hs, 0:DS], Act.Square,
                                 bias=0.0, scale=1.0)
            nc.vector.tensor_reduce(c2[:, hs], csq[:, hs, :], axis=AX,
                                    op=Alu.add)
            nc.vector.tensor_scalar(out=cbf[:, hs, DS:DS + 1], in0=c2[:, hs],
                                    scalar1=-0.5, scalar2=None, op0=Alu.mult)
            nc.vector.tensor_scalar(out=cbf[:, hs, 64 + DS:64 + DS + 1],
                                    in0=c2[:, hs], scalar1=-0.5, scalar2=None,
                                    op0=Alu.mult)
            # xbar transpose: out[p, i, c] = in[c, i*128 + p]
            dma.dma_start_transpose(cT[:, hs, :], cbf[:, hs, :])

        def x_chunk(k):
            cs = slice(k * XB, (k + 1) * XB)
            pp = slice(k * XB // 2, (k + 1) * XB // 2)
            nc.scalar.copy(xbf[:, cs, 0:DS], xsb[:, cs, 0:DS])
            nc.scalar.activation(xsq[:, cs, :], xsb[:, cs, 0:DS], Act.Square,
                                 bias=0.0, scale=1.0)
            nc.vector.tensor_reduce(x2[:, pp, :], xsq[:, cs, :], axis=AX,
                                    op=Alu.add)
            # cap = 0.5*x2 - THRESH/2 ; ncb = MARGIN - cap
            nc.vector.tensor_scalar(out=cap[:, pp, :], in0=x2[:, pp, :],
                                    scalar1=0.5, scalar2=-THRESH / 2,
                                    op0=Alu.mult, op1=Alu.add)
            nc.vector.tensor_scalar(out=ncb[:, pp, :], in0=x2[:, pp, :],
                                    scalar1=-0.5, scalar2=MARGIN + THRESH / 2,
                                    op0=Alu.mult, op1=Alu.add)
            # [128, 16*64] -> pairs: rows 0:64 even block, 64:128 odd block
            dma.dma_start_transpose(xT[:, pp, :], xbf[:, cs, :])

        codes_half(0)
        codes_half(1)
        x_chunk(0)
        codes_half(2)
        codes_half(3)
        for k in range(1, XCH):
            nc.scalar.dma_start(out=xsb[:, k * XB:(k + 1) * XB, :],
                                in_=x_r[:, k * XB:(k + 1) * XB, :])
        for k in range(1, XCH):
            x_chunk(k)

        # separate PSUM pools per drain engine: a shared tile would chain the
        # two readers (ACT waits for DVE), serializing the drains
        with tc.tile_pool(name="gd", bufs=2, space="PSUM") as pd, \
             tc.tile_pool(name="ga", bufs=2, space="PSUM") as pa:
            for pr in range(NPAIR):
                for q in range(4):
                    gtd = pd.tile((128, 1024), fp32, tag="gd",
                                  name=f"gd{pr}_{q}")
                    gta = pa.tile((128, 1024), fp32, tag="ga",
                                  name=f"ga{pr}_{q}")
                    for c in range(2):
                        ch = 2 * q + c
                        # the two K=64 PE tiles run the even (rows 0:64) and
                        # odd (rows 64:128) blocks of this pair concurrently
                        nc.tensor.matmul(
                            gtd[:, c * 512:(c + 1) * 512],
                            xT[0:64, pr, :], cT[0:64, 4 * ch:4 * ch + 4, :],
                            start=True, stop=True)
                        nc.tensor.matmul(
                            gta[:, c * 512:(c + 1) * 512],
                            xT[64:128, pr, :], cT[64:128, 4 * ch:4 * ch + 4, :],
                            start=True, stop=True)
                    idx = 4 * pr + q
                    nc.vector.tensor_reduce(gmax[:, pr, q:q + 1],
                                            gtd, axis=AX, op=Alu.max)
                    nc.scalar.activation(scr, gta, Act.Relu,
                                         bias=ncb[:, pr, 1:2], scale=1.0,
                                         accum_out=rsum[:, idx:idx + 1])

        # finalize. Even blocks: h = max_q gmax - cap; odd blocks: relu sums.
        nc.vector.tensor_reduce(gm2, gmax, axis=AX, op=Alu.max)
        nc.vector.tensor_tensor(gm2, gm2, cap[:, :, 0], op=Alu.subtract)
        nc.vector.tensor_reduce(fin[:, 0:1], gm2, axis=AX, op=Alu.max)
        nc.scalar.activation(scr[:, 0:NPAIR * 4], rsum, Act.Copy, bias=0.0,
                             scale=1.0, accum_out=fin[:, 1:2])
        # per-partition-contiguous output: o_d[2p] = hmax_p, o_d[2p+1] = rsum_p
        dma.dma_start(out=o_d[:].rearrange("(p c) -> p c", p=128), in_=fin)

    nc.compile()
    return nc


def _build(stage=6):
    """Exact argmin fallback (identical to the original full program)."""
    import concourse.bacc as bacc
    import concourse.mybir as mybir
    import concourse.tile as tile
    from contextlib import ExitStack

    fp32 = mybir.dt.float32
    bf16 = mybir.dt.bfloat16
    u32 = mybir.dt.uint32
    Alu = mybir.AluOpType
    Act = mybir.ActivationFunctionType

    nc = bacc.Bacc(
        "TRN2",
        target_bir_lowering=False,
        debug=False,
        enable_asserts=False,
        num_devices=1,
    )

    x_d = nc.dram_tensor("x", (TOK, D), fp32, kind="ExternalInput")
    c_d = nc.dram_tensor("codes", (M, D), fp32, kind="ExternalInput")
    id_d = nc.dram_tensor("ident", (128, 128), fp32, kind="ExternalInput")
    o_d = nc.dram_tensor("out", (TOK,), u32, kind="ExternalOutput")

    with tile.TileContext(nc) as tc, ExitStack() as ctx:
        sb = ctx.enter_context(tc.tile_pool(name="sb", bufs=1))

        ident = sb.tile((128, 128), fp32, tag="ident")
        xsb = sb.tile((128, NBLK, D), fp32, tag="xsb")
        csb = sb.tile((128, CBLK, D), fp32, tag="csb")
        xT = sb.tile((65, NBLK * 128), bf16, tag="xT")
        cT = sb.tile((65, M), bf16, tag="cT")
        cTsq = sb.tile((64, M), bf16, tag="cTsq")
        ones64 = sb.tile((64, 1), bf16, tag="ones64")
        x2 = sb.tile((128, NBLK), fp32, tag="x2")
        sq_all = sb.tile((128, NBLK, D), fp32, tag="sq_all")
        out_sb = sb.tile((128, NBLK), u32, tag="out_sb")
        top8 = sb.tile((128, 8), bf16, tag="top8")
        idx8 = sb.tile((128, 8), u32, tag="idx8")
        gmaxf = sb.tile((128, 1), fp32, tag="gmaxf")
        mind2 = sb.tile((128, 1), fp32, tag="mind2")
        mask = sb.tile((128, 1), mybir.dt.uint8, tag="mask")

        dma = nc.default_dma_engine
        dma.dma_start(out=ident, in_=id_d[:, :])
        dma.dma_start(out=xsb, in_=x_d[:, :].rearrange("(b p) d -> p b d", p=128))
        dma.dma_start(out=csb, in_=c_d[:, :].rearrange("(b p) d -> p b d", p=128))

        nc.vector.memset(xT[64:65, :], 1.0)
        nc.vector.memset(ones64, 1.0)
        nc.vector.memset(out_sb, 0xFFFFFFFF)

        if stage >= 2:
            with tc.tile_pool(name="tpsum", bufs=4, space="PSUM") as tp:
                for cb in range(CBLK):
                    pt = tp.tile((64, 128), fp32, tag="ct")
                    nc.tensor.transpose(pt, csb[:, cb, :], ident)
                    nc.scalar.copy(cT[0:64, cb * 128:(cb + 1) * 128], pt)
                for xb in range(NBLK):
                    pt = tp.tile((64, 128), fp32, tag="xt")
                    nc.tensor.transpose(pt, xsb[:, xb, :], ident)
                    nc.scalar.copy(xT[0:64, xb * 128:(xb + 1) * 128], pt)

            nc.vector.tensor_tensor(cTsq, cT[0:64, :], cT[0:64, :], op=Alu.mult)
            with tc.tile_pool(name="c2psum", bufs=2, space="PSUM") as cp:
                for j in range(NCH):
                    pt = cp.tile((1, 512), fp32, tag="c2")
                    nc.tensor.matmul(pt, ones64, cTsq[:, j * 512:(j + 1) * 512],
                                     start=True, stop=True)
                    nc.scalar.activation(cT[64:65, j * 512:(j + 1) * 512], pt,
                                         Act.Copy, bias=0.0, scale=-0.5)

        if stage >= 3:
            nc.scalar.activation(sq_all, xsb, Act.Square, bias=0.0, scale=1.0)
            nc.vector.tensor_reduce(x2, sq_all, axis=mybir.AxisListType.X,
                                    op=Alu.add)
        else:
            nc.vector.memset(x2, 1.0)

        if stage >= 4:
            with tc.tile_pool(name="gpsum", bufs=1, space="PSUM") as gp, \
                 tc.tile_pool(name="gsb", bufs=2) as gsb_pool:
                gbanks = [gp.tile((128, 512), fp32, tag=f"g{j}", name=f"g{j}")
                          for j in range(NCH)]
                for blk in range(NBLK):
                    lhsT = xT[:, blk * 128:(blk + 1) * 128]
                    g_sb = gsb_pool.tile((128, M), bf16, tag="g_sb")
                    for j in range(NCH):
                        nc.tensor.matmul(gbanks[j], lhsT,
                                         cT[:, j * 512:(j + 1) * 512],
                                         start=True, stop=True)
                        nc.scalar.copy(g_sb[:, j * 512:(j + 1) * 512], gbanks[j])
                    if stage >= 5:
                        nc.vector.max(top8, g_sb)
                        nc.vector.max_index(idx8, top8, g_sb)
                        nc.vector.tensor_copy(gmaxf, top8[:, 0:1])
                    if stage >= 6:
                        nc.vector.tensor_scalar(
                            out=mind2, in0=x2[:, blk:blk + 1],
                            scalar1=gmaxf[:, 0:1], scalar2=gmaxf[:, 0:1],
                            op0=Alu.subtract, op1=Alu.subtract)
                        nc.vector.tensor_scalar(
                            out=mask, in0=mind2, scalar1=THRESH, scalar2=None,
                            op0=Alu.is_le)
                        nc.vector.copy_predicated(out_sb[:, blk:blk + 1], mask,
                                                  idx8[:, 0:1])

        dma.dma_start(out=o_d[:].rearrange("(b p) -> p b", p=128), in_=out_sb)

    nc.compile()
    return nc


def _run(nc, in_maps, trace):
    from concourse import bass_utils
    try:
        return bass_utils.run_bass_kernel_spmd(
            nc, in_maps, list(range(NCORES)), trace=trace)
    except Exception:
        if not trace:
            raise
        return bass_utils.run_bass_kernel_spmd(
            nc, in_maps, list(range(NCORES)), trace=False)


def _run_full(x, codes, xf, trace):
    ident = np.eye(128, dtype=np.float32)
    in_maps = [
        {"x": xf[c], "codes": codes, "ident": ident}
        for c in range(NCORES)
    ]
    if "full" not in _CACHE:
        _CACHE["full"] = _build(6)
    res = _CACHE["last_res"] = _run(_CACHE["full"], in_maps, trace)
    out = np.concatenate(
        [np.asarray(res.results[c]["out"], dtype=np.uint32)
         for c in range(NCORES)])
    return out.reshape(B, N).view(np.int32)


def kernel(x: np.ndarray, codes: np.ndarray) -> np.ndarray:
    os.environ.setdefault("NEURON_RT_RESET_CORES", "1")
    x = np.ascontiguousarray(x, dtype=np.float32)
    codes = np.ascontiguousarray(codes, dtype=np.float32)
    xf = x.reshape(NCORES, TOK, D)
    trace = bool(os.environ.get("KERNEL_TRACE"))

    if os.environ.get("KERNEL_FORCE_FULL"):
        return _run_full(x, codes, xf, trace)

    in_maps = [{"x": xf[c], "codes": codes} for c in range(NCORES)]
    if "screen" not in _CACHE:
        _CACHE["screen"] = _build_screen()
    res = _CACHE["last_res"] = _run(_CACHE["screen"], in_maps, trace)

    all_clear = True
    for c in range(NCORES):
        s = np.asarray(res.results[c]["screen"], dtype=np.float32).reshape(128, 2)
        hmax, relusum = s[:, 0], s[:, 1]
        # certified: computed h < -(MARGIN/2) covers all quantization error
        if hmax.max() >= -MARGIN / 2 or relusum.max() > 0.0:
            all_clear = False
            break
    if all_clear:
        return np.full((B, N), -1, dtype=np.int32)

    return _run_full(x, codes, xf, trace)


# revision 25
# speedup vs baseline: 1.1712x; 1.1712x over previous
"""Nearest-neighbor tokenizer on 8 Trainium2 NeuronCores.

Math: d2[t,m] = ||x_t||^2 + ||c_m||^2 - 2 x_t.c_m over 65536 tokens x 4096 codes.
out[t] = argmin_m d2 if min d2 <= 0.1 else -1.

Fast path ("screen" program): per token, mind2 = 0.1 - 2*max_m h[t,m] where
h = x.c - ||c||^2/2 - cap_t and cap_t = (||x_t||^2 - 0.1)/2 (over the first DS
dims; partial d2 <= d2 keeps it sound). All-tokens-miss is certified by
max h < 0 with margin. Token blocks are processed in even/odd pairs computed
concurrently by the two 64-row PE tiles into separate PSUM pools; each fp32 is
read exactly once: DVE fused-max-reduces the even block's banks while ACT
fused-relu(+margin)-sums (accum_out) the odd block's banks in parallel. Host
checks the 2x128 per-core outputs; if any token might be within threshold,
falls back to the exact argmin program.

Sharding: data-parallel over tokens. Core c gets batches [2c, 2c+2) ->
a contiguous slab of 8192 tokens; the codebook is replicated.
"""

import os

import numpy as np

B, N, D = 16, 4096, 64
M = 4096
NCORES = 8
TOK = B * N // NCORES          # 8192 tokens per core
NBLK = TOK // 128              # 64 blocks of 128 tokens
NCH = M // 512                 # 8 chunks of 512 codes
CBLK = M // 128                # 32 code blocks
THRESH = 0.1
DS = 63                        # screen dims (63 + appended row = K=64 tiles)
MARGIN = 2.0                   # screen slack (>> bf16 quantization error ~0.6)
FALLBACK_MARGIN = 2.0

_CACHE = {}


def _build_screen():
    """mind2 screen over the first DS dims (partial d2 <= d2, so certifying
    partial mind2 > THRESH is sound). Output per core, interleaved per
    partition: screen[2p] = max over partition p's even-block tokens of
    (max_m g~ - cap), screen[2p+1] = sum of relu(g~ - cap + MARGIN) over its
    odd-block tokens. All-clear iff all screen[2p] < -1 and screen[2p+1] == 0.

    The PE on this part is pinned at the 1.2 GHz throttled clock (HAM never
    reaches K=8/8 even under 6.8us of continuous matmuls), so single matmuls
    would cost 218us. The contraction is kept at K=64 (63 dims + appended
    -c2/2 / ones row): the 128x128 xbar transpose packs an EVEN token block
    into xT rows 0:64 and the following ODD block into rows 64:128, and cT
    carries a duplicate of the codes at partitions 64:128, so the two 64-row
    PE tiles compute both blocks of a pair concurrently. Even blocks are
    screened by the DVE max-reduce, odd blocks by the ACT relu-sum — both
    sound."""
    import concourse.bacc as bacc
    import concourse.mybir as mybir
    import concourse.tile as tile
    from contextlib import ExitStack

    fp32 = mybir.dt.float32
    bf16 = mybir.dt.bfloat16
    Alu = mybir.AluOpType
    Act = mybir.ActivationFunctionType
    AX = mybir.AxisListType.X

    nc = bacc.Bacc(
        "TRN2",
        target_bir_lowering=False,
        debug=False,
        enable_asserts=False,
        num_devices=1,
    )

    x_d = nc.dram_tensor("x", (TOK, D), fp32, kind="ExternalInput")
    c_d = nc.dram_tensor("codes", (M, D), fp32, kind="ExternalInput")
    o_d = nc.dram_tensor("screen", (256,), fp32, kind="ExternalOutput")

    ACT_N = 1024               # ACT's per-group drain (2 of 4 PSUM banks)

    with tile.TileContext(nc) as tc, ExitStack() as ctx:
        sb = ctx.enter_context(tc.tile_pool(name="sb", bufs=1))

        NPAIR = NBLK // 2             # 32 even/odd block pairs
        xsb = sb.tile((128, NBLK, D), fp32, tag="xsb")
        csb = sb.tile((128, CBLK, D), fp32, tag="csb")
        # bf16 copies; x: 64 cols per block (63 dims + ones col); codes:
        # 128 cols per block (dims + -c2/2 col, duplicated at cols 64..)
        xbf = sb.tile((128, NBLK, 64), bf16, tag="xbf")
        cbf = sb.tile((128, CBLK, 128), bf16, tag="cbf")
        # xT[*, pr, t]: rows 0:64 = even block of pair pr, rows 64:128 = odd
        xT = sb.tile((128, NPAIR, 128), bf16, tag="xT")
        cT = sb.tile((128, CBLK, 128), bf16, tag="cT")
        xsq = sb.tile((128, NBLK, DS), bf16, tag="xsq")
        csq = sb.tile((128, CBLK, DS), bf16, tag="csq")
        x2 = sb.tile((128, NPAIR, 2), fp32, tag="x2")
        c2 = sb.tile((128, CBLK), fp32, tag="c2")
        cap = sb.tile((128, NPAIR, 2), fp32, tag="cap")  # (x2 - 0.1)/2
        ncb = sb.tile((128, NPAIR, 2), fp32, tag="ncb")  # MARGIN - cap
        gmax = sb.tile((128, NPAIR, 4), fp32, tag="gmax")
        rsum = sb.tile((128, NPAIR * 4), fp32, tag="rsum")
        scr = sb.tile((128, ACT_N), bf16, tag="scr")
        gm2 = sb.tile((128, NPAIR), fp32, tag="gm2")
        fin = sb.tile((128, 2), fp32, tag="fin")

        dma = nc.default_dma_engine
        XCH = 4                       # x pipeline chunks
        XB = NBLK // XCH              # blocks per chunk
        CH2 = CBLK // 2
        # Sync queue: codes halves + x chunk 0 (the setup critical path);
        # x chunks 1-3 issue from the ACT queue after its critical-path ops.
        # Contiguous loads (16KB/partition runs): partition p's "block" r is
        # token 64p+r (codes: 32p+r). The screen is order-invariant, so any
        # consistent token/code <-> (p, r) mapping works; this one makes the
        # DMA fully sequential instead of 256B-strided.
        c_r = c_d[:, :].rearrange("(p r) d -> p r d", p=128)
        x_r = x_d[:, :].rearrange("(p r) d -> p r d", p=128)
        CQ = CBLK // 4
        for h in range(4):
            dma.dma_start(out=csb[:, h * CQ:(h + 1) * CQ, :],
                          in_=c_r[:, h * CQ:(h + 1) * CQ, :])
        dma.dma_start(out=xsb[:, 0:XB, :], in_=x_r[:, 0:XB, :])

        nc.vector.memset(xbf[:, :, DS:DS + 1], 1.0)

        def codes_half(h):
            hs = slice(h * CQ, (h + 1) * CQ)
            nc.vector.tensor_copy(cbf[:, hs, 0:DS], csb[:, hs, 0:DS])
            nc.vector.tensor_copy(cbf[:, hs, 64:64 + DS], csb[:, hs, 0:DS])
            nc.scalar.activation(csq[:, hs, :], csb[:, hs, 0:DS], Act.Square,
                                 bias=0.0, scale=1.0)
            nc.vector.tensor_reduce(c2[:, hs], csq[:, hs, :], axis=AX,
                                    op=Alu.add)
            nc.vector.tensor_scalar(out=cbf[:, hs, DS:DS + 1], in0=c2[:, hs],
                                    scalar1=-0.5, scalar2=None, op0=Alu.mult)
            nc.vector.tensor_scalar(out=cbf[:, hs, 64 + DS:64 + DS + 1],
                                    in0=c2[:, hs], scalar1=-0.5, scalar2=None,
                                    op0=Alu.mult)
            # xbar transpose: out[p, i, c] = in[c, i*128 + p]
            dma.dma_start_transpose(cT[:, hs, :], cbf[:, hs, :])

        def x_chunk(k):
            cs = slice(k * XB, (k + 1) * XB)
            pp = slice(k * XB // 2, (k + 1) * XB // 2)
            nc.scalar.copy(xbf[:, cs, 0:DS], xsb[:, cs, 0:DS])
            nc.scalar.activation(xsq[:, cs, :], xsb[:, cs, 0:DS], Act.Square,
                                 bias=0.0, scale=1.0)
            nc.vector.tensor_reduce(x2[:, pp, :], xsq[:, cs, :], axis=AX,
                                    op=Alu.add)
            # cap = 0.5*x2 - THRESH/2 ; ncb = MARGIN - cap
            nc.vector.tensor_scalar(out=cap[:, pp, :], in0=x2[:, pp, :],
                                    scalar1=0.5, scalar2=-THRESH / 2,
                                    op0=Alu.mult, op1=Alu.add)
            nc.vector.tensor_scalar(out=ncb[:, pp, :], in0=x2[:, pp, :],
                                    scalar1=-0.5, scalar2=MARGIN + THRESH / 2,
                                    op0=Alu.mult, op1=Alu.add)
            # [128, 16*64] -> pairs: rows 0:64 even block, 64:128 odd block
            dma.dma_start_transpose(xT[:, pp, :], xbf[:, cs, :])

        codes_half(0)
        codes_half(1)
        x_chunk(0)
        codes_half(2)
        codes_half(3)
        for k in range(1, XCH):
            nc.scalar.dma_start(out=xsb[:, k * XB:(k + 1) * XB, :],
                                in_=x_r[:, k * XB:(k + 1) * XB, :])
        for k in range(1, XCH):
            x_chunk(k)

        # separate PSUM pools per drain engine: a shared tile would chain the
        # two readers (ACT waits for DVE), serializing the drains
        with tc.tile_pool(name="gd", bufs=2, space="PSUM") as pd, \
             tc.tile_pool(name="ga", bufs=2, space="PSUM") as pa:
            for pr in range(NPAIR):
                for q in range(4):
                    gtd = pd.tile((128, 1024), fp32, tag="gd",
                                  name=f"gd{pr}_{q}")
                    gta = pa.tile((128, 1024), fp32, tag="ga",
                                  name=f"ga{pr}_{q}")
                    for c in range(2):
                        ch = 2 * q + c
                        # the two K=64 PE tiles run the even (rows 0:64) and
                        # odd (rows 64:128) blocks of this pair concurrently
                        nc.tensor.matmul(
                            gtd[:, c * 512:(c + 1) * 512],
                            xT[0:64, pr, :], cT[0:64, 4 * ch:4 * ch + 4, :],
                            start=True, stop=True)
                        nc.tensor.matmul(
                            gta[:, c * 512:(c + 1) * 512],
                            xT[64:128, pr, :], cT[64:128, 4 * ch:4 * ch + 4, :],
                            start=True, stop=True)
                    idx = 4 * pr + q
                    nc.vector.tensor_reduce(gmax[:, pr, q:q + 1],
                                            gtd, axis=AX, op=Alu.max)
                    nc.scalar.activation(scr, gta, Act.Relu,
                                         bias=ncb[:, pr, 1:2], scale=1.0,
                                         accum_out=rsum[:, idx:idx + 1])

        # finalize. Even blocks: h = max_q gmax - cap; odd blocks: relu sums.
        nc.vector.tensor_reduce(gm2, gmax, axis=AX, op=Alu.max)
        nc.vector.tensor_tensor(gm2, gm2, cap[:, :, 0], op=Alu.subtract)
        nc.vector.tensor_reduce(fin[:, 0:1], gm2, axis=AX, op=Alu.max)
        nc.scalar.activation(scr[:, 0:NPAIR * 4], rsum, Act.Copy, bias=0.0,
                             scale=1.0, accum_out=fin[:, 1:2])
        # per-partition-contiguous output: o_d[2p] = hmax_p, o_d[2p+1] = rsum_p
        dma.dma_start(out=o_d[:].rearrange("(p c) -> p c", p=128), in_=fin)

    nc.compile()
    return nc


def _build(stage=6):
    """Exact argmin fallback (identical to the original full program)."""
    import concourse.bacc as bacc
    import concourse.mybir as mybir
    import concourse.tile as tile
    from contextlib import ExitStack

    fp32 = mybir.dt.float32
    bf16 = mybir.dt.bfloat16
    u32 = mybir.dt.uint32
    Alu = mybir.AluOpType
    Act = mybir.ActivationFunctionType

    nc = bacc.Bacc(
        "TRN2",
        target_bir_lowering=False,
        debug=False,
        enable_asserts=False,
        num_devices=1,
    )

    x_d = nc.dram_tensor("x", (TOK, D), fp32, kind="ExternalInput")
    c_d = nc.dram_tensor("codes", (M, D), fp32, kind="ExternalInput")
    id_d = nc.dram_tensor("ident", (128, 128), fp32, kind="ExternalInput")
    o_d = nc.dram_tensor("out", (TOK,), u32, kind="ExternalOutput")

    with tile.TileContext(nc) as tc, ExitStack() as ctx:
        sb = ctx.enter_context(tc.tile_pool(name="sb", bufs=1))

        ident = sb.tile((128, 128), fp32, tag="ident")
        xsb = sb.tile((128, NBLK, D), fp32, tag="xsb")
        csb = sb.tile((128, CBLK, D), fp32, tag="csb")
        xT = sb.tile((65, NBLK * 128), bf16, tag="xT")
        cT = sb.tile((65, M), bf16, tag="cT")
        cTsq = sb.tile((64, M), bf16, tag="cTsq")
        ones64 = sb.tile((64, 1), bf16, tag="ones64")
        x2 = sb.tile((128, NBLK), fp32, tag="x2")
        sq_all = sb.tile((128, NBLK, D), fp32, tag="sq_all")
        out_sb = sb.tile((128, NBLK), u32, tag="out_sb")
        top8 = sb.tile((128, 8), bf16, tag="top8")
        idx8 = sb.tile((128, 8), u32, tag="idx8")
        gmaxf = sb.tile((128, 1), fp32, tag="gmaxf")
        mind2 = sb.tile((128, 1), fp32, tag="mind2")
        mask = sb.tile((128, 1), mybir.dt.uint8, tag="mask")

        dma = nc.default_dma_engine
        dma.dma_start(out=ident, in_=id_d[:, :])
        dma.dma_start(out=xsb, in_=x_d[:, :].rearrange("(b p) d -> p b d", p=128))
        dma.dma_start(out=csb, in_=c_d[:, :].rearrange("(b p) d -> p b d", p=128))

        nc.vector.memset(xT[64:65, :], 1.0)
        nc.vector.memset(ones64, 1.0)
        nc.vector.memset(out_sb, 0xFFFFFFFF)

        if stage >= 2:
            with tc.tile_pool(name="tpsum", bufs=4, space="PSUM") as tp:
                for cb in range(CBLK):
                    pt = tp.tile((64, 128), fp32, tag="ct")
                    nc.tensor.transpose(pt, csb[:, cb, :], ident)
                    nc.scalar.copy(cT[0:64, cb * 128:(cb + 1) * 128], pt)
                for xb in range(NBLK):
                    pt = tp.tile((64, 128), fp32, tag="xt")
                    nc.tensor.transpose(pt, xsb[:, xb, :], ident)
                    nc.scalar.copy(xT[0:64, xb * 128:(xb + 1) * 128], pt)

            nc.vector.tensor_tensor(cTsq, cT[0:64, :], cT[0:64, :], op=Alu.mult)
            with tc.tile_pool(name="c2psum", bufs=2, space="PSUM") as cp:
                for j in range(NCH):
                    pt = cp.tile((1, 512), fp32, tag="c2")
                    nc.tensor.matmul(pt, ones64, cTsq[:, j * 512:(j + 1) * 512],
                                     start=True, stop=True)
                    nc.scalar.activation(cT[64:65, j * 512:(j + 1) * 512], pt,
                                         Act.Copy, bias=0.0, scale=-0.5)

        if stage >= 3:
            nc.scalar.activation(sq_all, xsb, Act.Square, bias=0.0, scale=1.0)
            nc.vector.tensor_reduce(x2, sq_all, axis=mybir.AxisListType.X,
                                    op=Alu.add)
        else:
            nc.vector.memset(x2, 1.0)

        if stage >= 4:
            with tc.tile_pool(name="gpsum", bufs=1, space="PSUM") as gp, \
                 tc.tile_pool(name="gsb", bufs=2) as gsb_pool:
                gbanks = [gp.tile((128, 512), fp32, tag=f"g{j}", name=f"g{j}")
                          for j in range(NCH)]
                for blk in range(NBLK):
                    lhsT = xT[:, blk * 128:(blk + 1) * 128]
                    g_sb = gsb_pool.tile((128, M), bf16, tag="g_sb")
                    for j in range(NCH):
                        nc.tensor.matmul(gbanks[j], lhsT,
                                         cT[:, j * 512:(j + 1) * 512],
                                         start=True, stop=True)
                        nc.scalar.copy(g_sb[:, j * 512:(j + 1) * 512], gbanks[j])
                    if stage >= 5:
                        nc.vector.max(top8, g_sb)
                        nc.vector.max_index(idx8, top8, g_sb)
                        nc.vector.tensor_copy(gmaxf, top8[:, 0:1])
                    if stage >= 6:
                        nc.vector.tensor_scalar(
                            out=mind2, in0=x2[:, blk:blk + 1],
                            scalar1=gmaxf[:, 0:1], scalar2=gmaxf[:, 0:1],
                            op0=Alu.subtract, op1=Alu.subtract)
                        nc.vector.tensor_scalar(
                            out=mask, in0=mind2, scalar1=THRESH, scalar2=None,
                            op0=Alu.is_le)
                        nc.vector.copy_predicated(out_sb[:, blk:blk + 1], mask,
                                                  idx8[:, 0:1])

        dma.dma_start(out=o_d[:].rearrange("(b p) -> p b", p=128), in_=out_sb)

    nc.compile()
    return nc


def _run(nc, in_maps, trace):
    from concourse import bass_utils
    try:
        return bass_utils.run_bass_kernel_spmd(
            nc, in_maps, list(range(NCORES)), trace=trace)
    except Exception:
        if not trace:
            raise
        return bass_utils.run_bass_kernel_spmd(
            nc, in_maps, list(range(NCORES)), trace=False)


def _run_full(x, codes, xf, trace):
    ident = np.eye(128, dtype=np.float32)
    in_maps = [
        {"x": xf[c], "codes": codes, "ident": ident}
        for c in range(NCORES)
    ]
    if "full" not in _CACHE:
        _CACHE["full"] = _build(6)
    res = _CACHE["last_res"] = _run(_CACHE["full"], in_maps, trace)
    out = np.concatenate(
        [np.asarray(res.results[c]["out"], dtype=np.uint32)
         for c in range(NCORES)])
    return out.reshape(B, N).view(np.int32)


def kernel(x: np.ndarray, codes: np.ndarray) -> np.ndarray:
    os.environ.setdefault("NEURON_RT_RESET_CORES", "1")
    x = np.ascontiguousarray(x, dtype=np.float32)
    codes = np.ascontiguousarray(codes, dtype=np.float32)
    xf = x.reshape(NCORES, TOK, D)
    trace = bool(os.environ.get("KERNEL_TRACE"))

    if os.environ.get("KERNEL_FORCE_FULL"):
        return _run_full(x, codes, xf, trace)

    in_maps = [{"x": xf[c], "codes": codes} for c in range(NCORES)]
    if "screen" not in _CACHE:
        _CACHE["screen"] = _build_screen()
    res = _CACHE["last_res"] = _run(_CACHE["screen"], in_maps, trace)

    all_clear = True
    for c in range(NCORES):
        s = np.asarray(res.results[c]["screen"], dtype=np.float32).reshape(128, 2)
        hmax, relusum = s[:, 0], s[:, 1]
        # certified: computed h < -(MARGIN/2) covers all quantization error
        if hmax.max() >= -MARGIN / 2 or relusum.max() > 0.0:
            all_clear = False
            break
    if all_clear:
        return np.full((B, N), -1, dtype=np.int32)

    return _run_full(x, codes, xf, trace)
